# revision 4
# baseline (speedup 1.0000x reference)
"""Trainium2 Bass kernel for AudioGRUModel: GRU over 256 steps, final hidden.

Strategy: 8-way data-parallel over batch (32 rows/core), weights replicated.
All on-chip layouts are transposed ([feature-dim on partitions, batch on free])
so the sequential recurrence needs no per-step transposes.

Phase 1 (input projection): gi^T[3H, (s,b)] = W_ih.T^T @ x^T as a batched GEMM
over all (step, batch) columns in float32r (full-rate PE streaming at N=512),
with b_ih (+ b_hh for the r/z gates) folded in via the scalar engine's
per-partition bias during PSUM evacuation. Result spilled to DRAM scratch.

Phase 2 (recurrence): per step, gh^T = W_hh.T^T @ h^T with W_hh.T stationary in
bf16 (FWL doubles the weight-load rate, which is the per-step floor), h cast to
bf16 for the matmul only; fp32 PSUM and gate math. r/z and n live in separate
PSUM banks so the r/z sigmoid overlaps the n-gate matmuls. b_hh's n-slice is
injected by a tiny K=4 selector matmul that doubles as the n-bank's start=True.
"""

import numpy as np
import ml_dtypes

import concourse.bass as bass
import concourse.tile as tile
from concourse import mybir, bacc
from concourse.bass_utils import run_bass_kernel_spmd

F32 = mybir.dt.float32
F32R = mybir.dt.float32r
BF16 = mybir.dt.bfloat16

B, INP, S, H = 256, 512, 256, 512
G3 = 3 * H            # 1536
NC = 8
BL = B // NC          # 32 batch rows per core
KC = H // 128         # 4 contraction chunks
MC = G3 // 128        # 12 output chunks (0-3 r, 4-7 z, 8-11 n)
SQ = 64               # steps per input-projection slab
SG = 16               # steps per 512-col matmul group (16*32=512)


def _build(steps=S, trace=False):
    nc = bacc.Bacc("TRN2", target_bir_lowering=False, debug=False)

    x_d = nc.dram_tensor("x", [BL, INP, steps], F32, kind="ExternalInput")
    wih_d = nc.dram_tensor("wih_t", [INP, G3], F32, kind="ExternalInput")
    whh_d = nc.dram_tensor("whh_t", [H, G3], BF16, kind="ExternalInput")
    bsum_d = nc.dram_tensor("bsum", [128, MC], F32, kind="ExternalInput")
    bhhn_d = nc.dram_tensor("bhhn", [KC, 128], BF16, kind="ExternalInput")
    sel4_d = nc.dram_tensor("sel4", [KC, 128], BF16, kind="ExternalInput")
    out_d = nc.dram_tensor("h_out", [BL, H], F32, kind="ExternalOutput")

    with tile.TileContext(nc) as tc:
        with (
            tc.tile_pool(name="consts", bufs=1) as consts,
            tc.tile_pool(name="dram", bufs=1, space="DRAM") as dram,
            tc.tile_pool(name="xstage", bufs=2) as xstage,
            tc.tile_pool(name="ipsum", bufs=4, space="PSUM") as ipsum,
            tc.tile_pool(name="evac", bufs=4) as evacp,
            tc.tile_pool(name="gload", bufs=4) as gload,
            tc.tile_pool(name="prz", bufs=2, space="PSUM") as przp,
            tc.tile_pool(name="pn", bufs=2, space="PSUM") as pnp,
            tc.tile_pool(name="gates", bufs=2) as gates,
        ):
            # ---- constants / weights ----
            wih = consts.tile([128, KC, G3], F32R)
            for k in range(KC):
                nc.sync.dma_start(out=wih[:, k, :],
                                  in_=wih_d[128 * k:128 * (k + 1), :].bitcast(F32R))
            whh = consts.tile([128, KC, G3], BF16)
            for k in range(KC):
                nc.sync.dma_start(out=whh[:, k, :], in_=whh_d[128 * k:128 * (k + 1), :])
            bsum = consts.tile([128, MC], F32)
            nc.sync.dma_start(out=bsum[:], in_=bsum_d.ap())
            bhhn = consts.tile([KC, 128], BF16)
            nc.sync.dma_start(out=bhhn[:], in_=bhhn_d.ap())
            sel4 = consts.tile([KC, 128], BF16)
            nc.sync.dma_start(out=sel4[:], in_=sel4_d.ap())

            h32 = consts.tile([128, 128], F32)    # h^T: [p, 32k+b] = h[b, 128k+p]
            nc.vector.memset(h32[:], 0.0)
            hbf = consts.tile([128, 128], BF16)
            nc.vector.memset(hbf[:], 0.0)

            gi_d = dram.tile([steps, 128, MC * 32], F32)  # gi^T scratch

            # ---- phase 1: input projection gi^T = W_ih^T x (+ biases) ----
            for q in range((steps + SQ - 1) // SQ):
                s0 = q * SQ
                sq = min(SQ, steps - s0)
                xt = xstage.tile([128, KC, BL, SQ], F32R)
                for k in range(KC):
                    nc.sync.dma_start(
                        out=xt[:, k, :, :sq],
                        in_=x_d[:, 128 * k:128 * (k + 1), s0:s0 + sq]
                        .rearrange("b p s -> p b s").bitcast(F32R),
                    )
                for g in range(sq // SG):
                    for m in range(MC):
                        ps = ipsum.tile([128, SG * BL], F32)
                        for k in range(KC):
                            rhs = (
                                xt[:, k, :, SG * g:SG * (g + 1)]
                                .rearrange("p b s -> p s b")
                            )
                            nc.tensor.matmul(
                                ps[:],
                                wih[:, k, 128 * m:128 * (m + 1)],
                                rhs,
                                start=(k == 0),
                                stop=(k == KC - 1),
                            )
                        ev = evacp.tile([128, SG * BL], F32)
                        nc.scalar.activation(
                            ev[:], ps[:], mybir.ActivationFunctionType.Identity,
                            bias=bsum[:, m:m + 1], scale=1.0,
                        )
                        nc.sync.dma_start(
                            out=gi_d[s0 + SG * g:s0 + SG * (g + 1), :, 32 * m:32 * (m + 1)]
                            .rearrange("s p b -> p s b"),
                            in_=ev.rearrange("p (s b) -> p s b", s=SG),
                        )

            # ---- phase 2: recurrence ----
            for t in range(steps):
                G = gload.tile([128, MC * 32], F32)
                nc.sync.dma_start(out=G[:], in_=gi_d[t])

                p_rz = przp.tile([128, 256], F32)
                p_n = pnp.tile([128, 128], F32)
                # n-gate bias via selector matmul; doubles as start=True for p_n
                nc.tensor.matmul(p_n[:], bhhn[:], sel4[:], start=True, stop=False)
                for m in range(MC):
                    out_ap = p_rz[:, 32 * m:32 * (m + 1)] if m < 8 else \
                        p_n[:, 32 * (m - 8):32 * (m - 7)]
                    for k in range(KC):
                        nc.tensor.matmul(
                            out_ap,
                            whh[:, k, 128 * m:128 * (m + 1)],
                            hbf[:, 32 * k:32 * (k + 1)],
                            start=(m == 0 and k == 0) if m < 8 else False,
                            stop=(k == KC - 1) and (m in (7, MC - 1)),
                        )

                s1 = gates.tile([128, 256], F32)
                nc.vector.tensor_add(s1[:], p_rz[:], G[:, 0:256])
                rz = gates.tile([128, 256], F32)
                nc.scalar.activation(rz[:], s1[:], mybir.ActivationFunctionType.Sigmoid)
                tt = gates.tile([128, 128], F32)
                nc.vector.tensor_mul(tt[:], rz[:, 0:128], p_n[:])
                vv = gates.tile([128, 128], F32)
                nc.vector.tensor_add(vv[:], tt[:], G[:, 256:384])
                nn_ = gates.tile([128, 128], F32)
                nc.scalar.activation(nn_[:], vv[:], mybir.ActivationFunctionType.Tanh)
                f1 = gates.tile([128, 128], F32)
                nc.vector.tensor_sub(f1[:], h32[:], nn_[:])
                f2 = gates.tile([128, 128], F32)
                nc.vector.tensor_mul(f2[:], rz[:, 128:256], f1[:])
                nc.vector.tensor_add(h32[:], nn_[:], f2[:])
                nc.vector.tensor_copy(hbf[:], h32[:])

            # ---- output: un-transpose h^T -> h ----
            for k in range(KC):
                nc.sync.dma_start(
                    out=out_d[:, 128 * k:128 * (k + 1)].rearrange("b p -> p b"),
                    in_=h32[:, 32 * k:32 * (k + 1)],
                )

    nc.compile()
    return nc


def _prep_inputs(x, weight_ih, weight_hh, bias_ih, bias_hh):
    x = np.ascontiguousarray(np.asarray(x, dtype=np.float32))
    w_ih = np.asarray(weight_ih, dtype=np.float32)
    w_hh = np.asarray(weight_hh, dtype=np.float32)
    b_ih = np.asarray(bias_ih, dtype=np.float32)
    b_hh = np.asarray(bias_hh, dtype=np.float32)

    wih_t = np.ascontiguousarray(w_ih.T)                               # [INP, 3H] f32
    whh_t = np.ascontiguousarray(w_hh.T).astype(ml_dtypes.bfloat16)    # [H, 3H] bf16
    # per-partition bias for evac: b_ih everywhere, + b_hh on r/z chunks
    bsum = np.empty((128, MC), np.float32)
    for m in range(MC):
        seg = b_ih[128 * m:128 * (m + 1)].copy()
        if m < 8:
            seg += b_hh[128 * m:128 * (m + 1)]
        bsum[:, m] = seg
    bhhn = b_hh[2 * H:].reshape(KC, 128).astype(ml_dtypes.bfloat16)
    sel4 = np.zeros((KC, 128), np.float32)
    for k in range(KC):
        sel4[k, 32 * k:32 * (k + 1)] = 1.0
    sel4 = sel4.astype(ml_dtypes.bfloat16)

    shared = {"wih_t": wih_t, "whh_t": whh_t, "bsum": bsum,
              "bhhn": bhhn, "sel4": sel4}
    in_maps = []
    for c in range(NC):
        m = dict(shared)
        m["x"] = np.ascontiguousarray(x[BL * c:BL * (c + 1)])
        in_maps.append(m)
    return in_maps


_NC_CACHE = {}


def _get_nc(steps=S):
    if steps not in _NC_CACHE:
        _NC_CACHE[steps] = _build(steps)
    return _NC_CACHE[steps]


def kernel(x, weight_ih, weight_hh, bias_ih, bias_hh):
    nc = _get_nc(S)
    in_maps = _prep_inputs(x, weight_ih, weight_hh, bias_ih, bias_hh)
    res = run_bass_kernel_spmd(nc, in_maps, core_ids=list(range(NC)))
    return np.concatenate(
        [np.asarray(res.results[c]["h_out"]) for c in range(NC)], axis=0
    ).astype(np.float32)
